# revision 45
# baseline (speedup 1.0000x reference)
"""Trainium2 Bass kernel for nn_CrossDimensionalAttention_60550448939365.

Math reduction (see reference): scores[b,i,j] = tp[b,i] . fp[b] is constant in
j, so softmax over j is exactly uniform and attended[b,i,:] = fp[b,:].  The
whole Wt/scores/softmax/bmm pipeline is a no-op.  What remains:

    z   = x + fp[b]                 (broadcast over seq)
    y   = LN1(z) @ W2 + c2          W2 = g1[:,None]*(Wo.T + I)
                                    c2 = b1 + bo + Wo @ b1
    out = LN2(y) * g2 + b2

Fast path (c2 == 0, g2 == 1, b2 == 0 -- true for this checkpoint):
LN1's 1/sd row scale cancels inside LN2 (scale invariance), and the row-mean
centering folds into the weights:

    (z - rowmean(z)) @ W2 = z @ W2c,   W2c = W2 - ones * colsum(W2)/H
    out = LN2(x @ W2c + fp @ W2c)

so the device does NO LN1 work at all: cast x to bf16, transpose via PE,
matmul with host-precast bf16 W2c (plus two rank-1 terms adding the constant
fp@W2c row in hi+lo bf16 halves), then LN2 stats + apply, store.  A general
program (the original full pipeline) is built when the fast-path conditions
don't hold, so kernel() is correct for any inputs.

Sharding: rows of flattened [B*S, H] = [8192, 512] split evenly across the 8
cores (1024 rows each, each shard entirely within one batch b = core//2).
"""

import os
import numpy as np

import concourse.bass as bass
import concourse.tile as tile
from concourse import bacc, mybir
from concourse.bass_utils import run_bass_kernel_spmd
from concourse.masks import make_identity

H = 512
B = 4
S = 2048
N_CORES = 8
ROWS = (B * S) // N_CORES  # 1024 rows per core
P = 128
NT = ROWS // P             # 8 token tiles per core
EPS = 1e-5

F32 = mybir.dt.float32
F32R = mybir.dt.float32r
BF16 = mybir.dt.bfloat16
AF = mybir.ActivationFunctionType
ALU = mybir.AluOpType


def _row_ap(src: bass.AP) -> bass.AP:
    """View a [N]-shaped DRAM AP as [1, N]."""
    return bass.AP(tensor=src.tensor, offset=src.offset, ap=[[0, 1]] + list(src.ap))


def build_fast_program() -> bass.Bass:
    """out = LN2(x @ W2c + fprow); W2c/fprow host-precomputed, bf16.

    x is DMA'd with an f32r dtype tag (same bits as f32) so the PE can
    transpose it directly at 1 cycle/col; the f32r->bf16 rounding happens
    for free inside the PSUM-evacuation copies, which are split between
    ACT and DVE to balance their per-tile loads.
    """
    nc = bacc.Bacc("TRN2", target_bir_lowering=False, debug=False)

    x = nc.dram_tensor("x", [ROWS, H], F32R, kind="ExternalInput").ap()
    w2c = nc.dram_tensor("w2c", [H, H], BF16, kind="ExternalInput").ap()   # [h,k]
    fpb = nc.dram_tensor("fpb", [H], BF16, kind="ExternalInput").ap()
    out = nc.dram_tensor("out", [ROWS, H], F32, kind="ExternalOutput").ap()

    with tile.TileContext(nc) as tc:
        with (
            tc.tile_pool(name="consts", bufs=1) as consts,
            tc.tile_pool(name="xs", bufs=8) as xs,
            tc.tile_pool(name="xcts", bufs=3) as xcts,
            tc.tile_pool(name="xctv", bufs=3) as xctv,
            tc.tile_pool(name="stats", bufs=6) as stats,
            tc.tile_pool(name="smalls", bufs=12) as smalls,
            tc.tile_pool(name="outs", bufs=3) as outs,
            tc.tile_pool(name="psum_t", bufs=3, space="PSUM") as psum_t,
            tc.tile_pool(name="psum_y", bufs=3, space="PSUM") as psum_y,
            tc.tile_pool(name="psum_d", bufs=1, space="PSUM") as psum_d,
        ):
            # ---- x loads: prefetch all 8 virtual tiles on the sync(SP)
            # ring.  Virtual tile v (c=v//2, j=v%2) holds DRAM rows
            # {256c + 2p + j}: one 2KB descriptor per partition at 4KB
            # stride, so a later store of tiles (2c, 2c+1) as [128,2,512]
            # needs only one contiguous 4KB descriptor per partition.
            x_all = []
            w2cs = consts.tile([P, 4, H], BF16)
            for v in range(NT):
                c, j = v // 2, v % 2
                xt = xs.tile([P, H], F32R)
                nc.sync.dma_start(
                    out=xt,
                    in_=x[c * 2 * P:(c + 1) * 2 * P, :].rearrange(
                        "(p two) k -> p two k", two=2)[:, j, :],
                )
                x_all.append(xt)

            # ---- one-time constants on the scalar(ACT) ring so they don't
            # delay x loads on the sync ring.  fp row (tiny) first: the ring
            # is FIFO and the first C-stage needs it.
            fpb_row = consts.tile([1, H], BF16)
            nc.scalar.dma_start(out=fpb_row, in_=_row_ap(fpb))
            ones_bf = consts.tile([1, P], BF16)
            nc.vector.memset(ones_bf, 1.0)

            # Identity build comes before the SWDGE weight loads: both run
            # on the Q7 and the first transposes are gated on the identity.
            iden_f32 = consts.tile([P, P], F32)
            make_identity(nc, iden_f32)
            iden = consts.tile([P, P], F32R)
            nc.gpsimd.tensor_copy(iden, iden_f32)

            # Per-chunk weight loads, all through the GPSIMD SWDGE ring: the
            # Q7 generates descriptors serially (~1us apiece, in parallel
            # with the HWDGE rings), which staggers the chunk doorbells so
            # chunks 2/3 stop competing with the x2/x3 loads the pipeline
            # needs first, yet still land before C-stage h reaches them.
            for h in range(4):
                nc.gpsimd.dma_start(out=w2cs[:, h, :],
                                    in_=w2c[h * P:(h + 1) * P, :])

            epst = consts.tile([P, 1], F32)
            nc.vector.memset(epst, EPS)

            # Hoist the ACT Sqrt table load into the preamble.
            dsq = smalls.tile([P, 1], F32, tag="dsq")
            nc.scalar.activation(dsq, epst, AF.Sqrt, bias=0.0, scale=1.0)

            # PE warm-up burst: keeps the tensor engine active while x0 is
            # in flight so HAM has up-clocked it before the real transposes.
            dwm = psum_d.tile([P, P], F32, tag="dwm")
            for _ in range(16):
                nc.tensor.matmul(dwm, ones_bf, ones_bf, start=True, stop=True)
            # Dummy PE op absorbing the identity's GPSIMD wait.
            d1 = psum_d.tile([P, P], F32R, tag="dummy")
            nc.tensor.transpose(d1, iden, iden)

            # ---- software-pipelined main loop over 8 token tiles ----
            # B(t): PE transposes tile t; evac adds fp chunk bias and casts
            #       to bf16 (2 chunks on ACT, 2 on DVE)
            # C(t-1): W2c matmuls + LN2 stats/apply
            # store chunk c after C(2c+1), interleaved like the loads.
            xct_all, ot_ch = {}, {}
            for i in range(NT + 1):
                if i < NT:
                    t = i
                    xt = x_all[t]
                    ptr = psum_t.tile([P, 4, P], F32R)
                    for h in range(4):
                        nc.tensor.transpose(ptr[:, h, :], xt[:, h * P:(h + 1) * P],
                                            iden)
                    xct_a = xcts.tile([P, 2, P], BF16)
                    nc.scalar.copy(xct_a, ptr[:, 0:2, :])
                    xct_b = xctv.tile([P, 2, P], BF16)
                    nc.vector.tensor_copy(xct_b, ptr[:, 2:4, :])
                    xct_all[t] = (xct_a, xct_b)

                if i >= 1:
                    k = i - 1
                    xct_a, xct_b = xct_all[k]
                    py = psum_y.tile([P, H], F32)
                    nc.tensor.matmul(py, ones_bf, fpb_row, start=True, stop=False)
                    for h in range(4):
                        src = xct_a[:, h, :] if h < 2 else xct_b[:, h - 2, :]
                        nc.tensor.matmul(
                            py, src, w2cs[:, h, :],
                            start=False, stop=(h == 3),
                        )

                    st2 = stats.tile([P, 6], F32, tag="st")
                    nc.vector.bn_stats(st2, py)
                    mv2 = stats.tile([P, 2], F32, tag="mv")
                    nc.vector.bn_aggr(mv2, st2)
                    sd2 = smalls.tile([P, 1], F32, tag="sd")
                    nc.scalar.activation(sd2, mv2[:, 1:2], AF.Sqrt, bias=epst,
                                         scale=1.0)
                    s2 = smalls.tile([P, 1], F32, tag="s")
                    nc.vector.reciprocal(s2, sd2)
                    negms2 = smalls.tile([P, 1], F32, tag="negms")
                    # Last two tiles: keep the chain on DVE — the GPSIMD hop
                    # adds ~300ns of latency to the pipeline drain.
                    neg_eng = nc.vector if k >= NT - 2 else nc.gpsimd
                    neg_eng.tensor_scalar(
                        negms2, mv2[:, 0:1], s2, -1.0, op0=ALU.mult, op1=ALU.mult
                    )

                    c, j = k // 2, k % 2
                    if j == 0:
                        ot2 = outs.tile([P, 2, H], F32)
                        ot_ch[c] = ot2
                    nc.scalar.activation(ot_ch[c][:, j, :], py, AF.Identity,
                                         bias=negms2, scale=s2)
                    if j == 1:
                        nc.sync.dma_start(
                            out=out[c * 2 * P:(c + 1) * 2 * P, :].rearrange(
                                "(p two) k -> p two k", two=2),
                            in_=ot_ch[c],
                        )

    nc.compile()
    return nc


# ---------------------------------------------------------------------------
# General (slow, fully-correct) program: original pipeline with LN1.
# Used only when c2 != 0 or g2 != 1 or b2 != 0.
# ---------------------------------------------------------------------------

def _bcast_ap(src: bass.AP, parts: int) -> bass.AP:
    return bass.AP(tensor=src.tensor, offset=src.offset, ap=[[0, parts]] + list(src.ap))


def build_general_program() -> bass.Bass:
    nc = bacc.Bacc("TRN2", target_bir_lowering=False, debug=False)

    x = nc.dram_tensor("x", [ROWS, H], F32, kind="ExternalInput").ap()
    w2 = nc.dram_tensor("w2", [H, H], F32, kind="ExternalInput").ap()   # [h,k]
    c2 = nc.dram_tensor("c2", [H], F32, kind="ExternalInput").ap()
    fp = nc.dram_tensor("fp", [H], F32, kind="ExternalInput").ap()
    g2 = nc.dram_tensor("g2", [H], F32, kind="ExternalInput").ap()
    b2 = nc.dram_tensor("b2", [H], F32, kind="ExternalInput").ap()
    out = nc.dram_tensor("out", [ROWS, H], F32, kind="ExternalOutput").ap()

    MD = F32R

    with tile.TileContext(nc) as tc:
        with (
            tc.tile_pool(name="consts", bufs=1) as consts,
            tc.tile_pool(name="xs", bufs=4) as xs,
            tc.tile_pool(name="zs", bufs=4) as zs,
            tc.tile_pool(name="xns", bufs=8) as xns,
            tc.tile_pool(name="xnts", bufs=3) as xnts,
            tc.tile_pool(name="stats", bufs=6) as stats,
            tc.tile_pool(name="smalls", bufs=12) as smalls,
            tc.tile_pool(name="ts", bufs=3) as ts_pool,
            tc.tile_pool(name="outs", bufs=3) as outs,
            tc.tile_pool(name="psum_t", bufs=3, space="PSUM") as psum_t,
            tc.tile_pool(name="psum_y", bufs=3, space="PSUM") as psum_y,
            tc.tile_pool(name="psum_d", bufs=1, space="PSUM") as psum_d,
        ):
            ones1 = consts.tile([1, P], F32)
            nc.vector.memset(ones1, 1.0)
            onesmm = consts.tile([1, P], MD)
            nc.vector.tensor_copy(onesmm, ones1)

            fprow = consts.tile([1, H], F32)
            nc.sync.dma_start(out=fprow, in_=_row_ap(fp))
            fpmm = consts.tile([1, H], MD)
            nc.vector.tensor_copy(fpmm, fprow)
            fp_ps = psum_d.tile([P, H], F32, tag="bcast")
            nc.tensor.matmul(fp_ps, onesmm, fpmm, start=True, stop=True)
            fpb = consts.tile([P, H], F32)
            nc.scalar.copy(fpb, fp_ps)

            g2b = consts.tile([P, H], F32)
            nc.gpsimd.dma_start(out=g2b, in_=_bcast_ap(g2, P))
            b2b = consts.tile([P, H], F32)
            nc.gpsimd.dma_start(out=b2b, in_=_bcast_ap(b2, P))

            c2row = consts.tile([1, H], F32)
            nc.sync.dma_start(out=c2row, in_=_row_ap(c2))
            c2mm = consts.tile([1, H], MD)
            nc.vector.tensor_copy(c2mm, c2row)

            iden_f32 = consts.tile([P, P], F32)
            make_identity(nc, iden_f32)
            iden = consts.tile([P, P], F32R)
            nc.gpsimd.tensor_copy(iden, iden_f32)
            epst = consts.tile([P, 1], F32)
            nc.vector.memset(epst, EPS)

            d1 = psum_d.tile([P, P], MD, tag="dummy")
            nc.tensor.transpose(d1, iden, iden)

            xn_all, xnt_all = {}, {}
            w2mm = consts.tile([P, 4, H], MD)
            for i in range(NT + 3):
                if i == 1:
                    w2s = consts.tile([P, 4, H], F32)
                    nc.sync.dma_start(
                        out=w2s, in_=w2.rearrange("(t p) k -> p t k", p=P)
                    )
                    nc.scalar.copy(w2mm, w2s)

                if i < NT:
                    xt = xs.tile([P, H], F32)
                    nc.sync.dma_start(out=xt, in_=x[i * P:(i + 1) * P, :])

                    z = zs.tile([P, H], F32)
                    nc.vector.tensor_add(z, xt, fpb)

                    st1 = stats.tile([P, 6], F32, tag="st")
                    nc.vector.bn_stats(st1, z)
                    mv1 = stats.tile([P, 2], F32, tag="mv")
                    nc.vector.bn_aggr(mv1, st1)
                    sd1 = smalls.tile([P, 1], F32, tag="sd")
                    nc.scalar.activation(sd1, mv1[:, 1:2], AF.Sqrt, bias=epst,
                                         scale=1.0)
                    s1 = smalls.tile([P, 1], F32, tag="s")
                    nc.vector.reciprocal(s1, sd1)
                    negms1 = smalls.tile([P, 1], F32, tag="negms")
                    nc.vector.tensor_scalar(
                        negms1, mv1[:, 0:1], s1, -1.0, op0=ALU.mult, op1=ALU.mult
                    )
                    xn = xns.tile([P, H], MD)
                    nc.scalar.activation(xn, z, AF.Identity, bias=negms1, scale=s1)
                    xn_all[i] = xn

                if 2 <= i < NT + 2:
                    j = i - 2
                    xn = xn_all[j]
                    ptr = psum_t.tile([P, 4, P], MD)
                    for h in range(4):
                        nc.tensor.transpose(ptr[:, h, :], xn[:, h * P:(h + 1) * P],
                                            iden)
                    xnt = xnts.tile([P, 4, P], MD)
                    nc.scalar.copy(xnt, ptr)
                    xnt_all[j] = xnt

                if i >= 3:
                    k = i - 3
                    xnt = xnt_all[k]
                    py = psum_y.tile([P, H], F32)
                    nc.tensor.matmul(py, onesmm, c2mm, start=True, stop=False)
                    for h in range(4):
                        nc.tensor.matmul(
                            py, xnt[:, h, :], w2mm[:, h, :],
                            start=False, stop=(h == 3),
                        )

                    st2 = stats.tile([P, 6], F32, tag="st")
                    nc.vector.bn_stats(st2, py)
                    mv2 = stats.tile([P, 2], F32, tag="mv")
                    nc.vector.bn_aggr(mv2, st2)
                    sd2 = smalls.tile([P, 1], F32, tag="sd")
                    nc.scalar.activation(sd2, mv2[:, 1:2], AF.Sqrt, bias=epst,
                                         scale=1.0)
                    s2 = smalls.tile([P, 1], F32, tag="s")
                    nc.vector.reciprocal(s2, sd2)
                    negms2 = smalls.tile([P, 1], F32, tag="negms")
                    nc.vector.tensor_scalar(
                        negms2, mv2[:, 0:1], s2, -1.0, op0=ALU.mult, op1=ALU.mult
                    )

                    t = ts_pool.tile([P, H], F32)
                    nc.scalar.activation(t, py, AF.Identity, bias=negms2, scale=s2)

                    t2 = outs.tile([P, H], F32, tag="t2")
                    nc.gpsimd.tensor_mul(t2, t, g2b)
                    ot = outs.tile([P, H], F32, tag="ot")
                    nc.gpsimd.tensor_add(ot, t2, b2b)

                    nc.sync.dma_start(out=out[k * P:(k + 1) * P, :], in_=ot)

    nc.compile()
    return nc


def _weights(Wo, g1, b1, bo):
    f32 = np.float32
    W2 = g1[:, None] * (Wo.T + np.eye(H, dtype=f32))           # [h,k]
    c2 = b1 + bo + Wo @ b1                                     # [k]
    return W2, c2


def _host_prep(temporal_features, static_features, Wt, bt, Wf, bf, Wo, bo,
               g1, b1, g2, b2):
    f32 = np.float32
    x = np.ascontiguousarray(np.asarray(temporal_features, dtype=f32)).reshape(B * S, H)
    st = np.asarray(static_features, dtype=f32)
    Wf = np.asarray(Wf, dtype=f32)
    bf = np.asarray(bf, dtype=f32)
    Wo = np.asarray(Wo, dtype=f32)
    bo = np.asarray(bo, dtype=f32)
    g1 = np.asarray(g1, dtype=f32)
    b1 = np.asarray(b1, dtype=f32)
    g2 = np.asarray(g2, dtype=f32)
    b2 = np.asarray(b2, dtype=f32)

    fp = st @ Wf.T + bf                                        # [B,H]
    W2, c2 = _weights(Wo, g1, b1, bo)

    fast = (not np.any(c2 != 0.0)) and (not np.any(g2 != 1.0)) \
        and (not np.any(b2 != 0.0))

    in_maps = []
    if fast:
        import ml_dtypes
        bf16 = np.dtype(ml_dtypes.bfloat16)
        wcs = W2.sum(axis=0)                                   # [k]
        W2c = W2 - wcs[None, :] / f32(H)                       # centered cols
        w2c_bf = np.ascontiguousarray(W2c.astype(bf16))
        fpw2c = (fp @ W2c).astype(f32)                          # [B,H]
        for c in range(N_CORES):
            shard = np.ascontiguousarray(x[c * ROWS:(c + 1) * ROWS])
            row = fpw2c[(c * ROWS) // S]
            in_maps.append({
                "x": shard,
                "w2c": w2c_bf,
                "fpb": np.ascontiguousarray(row.astype(bf16)),
            })
        return True, in_maps

    for c in range(N_CORES):
        shard = np.ascontiguousarray(x[c * ROWS:(c + 1) * ROWS])
        in_maps.append({
            "x": shard,
            "w2": np.ascontiguousarray(W2),
            "c2": np.ascontiguousarray(c2),
            "fp": np.ascontiguousarray(fp[(c * ROWS) // S]),
            "g2": np.ascontiguousarray(g2),
            "b2": np.ascontiguousarray(b2),
        })
    return False, in_maps


_NC_CACHE = {}


def _get_program(fast: bool):
    if fast not in _NC_CACHE:
        _NC_CACHE[fast] = build_fast_program() if fast else build_general_program()
    return _NC_CACHE[fast]


def run(inputs: dict, trace: bool = False):
    """Returns (output [B,S,H] f32, BassKernelResults)."""
    fast, in_maps = _host_prep(**inputs)
    nc = _get_program(fast)
    res = run_bass_kernel_spmd(nc, in_maps, list(range(N_CORES)), trace=trace)
    shards = [res.results[c]["out"] for c in range(N_CORES)]
    full = np.concatenate(shards, axis=0).reshape(B, S, H).astype(np.float32)
    return full, res


def kernel(**inputs) -> np.ndarray:
    out, _ = run(inputs, trace=False)
    return out


# revision 46
# speedup vs baseline: 1.0589x; 1.0589x over previous
"""Trainium2 Bass kernel for nn_CrossDimensionalAttention_60550448939365.

Math reduction (see reference): scores[b,i,j] = tp[b,i] . fp[b] is constant in
j, so softmax over j is exactly uniform and attended[b,i,:] = fp[b,:].  The
whole Wt/scores/softmax/bmm pipeline is a no-op.  What remains:

    z   = x + fp[b]                 (broadcast over seq)
    y   = LN1(z) @ W2 + c2          W2 = g1[:,None]*(Wo.T + I)
                                    c2 = b1 + bo + Wo @ b1
    out = LN2(y) * g2 + b2

Fast path (c2 == 0, g2 == 1, b2 == 0 -- true for this checkpoint):
LN1's 1/sd row scale cancels inside LN2 (scale invariance), and the row-mean
centering folds into the weights:

    (z - rowmean(z)) @ W2 = z @ W2c,   W2c = W2 - ones * colsum(W2)/H
    out = LN2(x @ W2c + fp @ W2c)

so the device does NO LN1 work at all: cast x to bf16, transpose via PE,
matmul with host-precast bf16 W2c (plus two rank-1 terms adding the constant
fp@W2c row in hi+lo bf16 halves), then LN2 stats + apply, store.  A general
program (the original full pipeline) is built when the fast-path conditions
don't hold, so kernel() is correct for any inputs.

Sharding: rows of flattened [B*S, H] = [8192, 512] split evenly across the 8
cores (1024 rows each, each shard entirely within one batch b = core//2).
"""

import os
import numpy as np

import concourse.bass as bass
import concourse.tile as tile
from concourse import bacc, mybir
from concourse.bass_utils import run_bass_kernel_spmd
from concourse.masks import make_identity

H = 512
B = 4
S = 2048
N_CORES = 8
ROWS = (B * S) // N_CORES  # 1024 rows per core
P = 128
NT = ROWS // P             # 8 token tiles per core
EPS = 1e-5

F32 = mybir.dt.float32
F32R = mybir.dt.float32r
BF16 = mybir.dt.bfloat16
AF = mybir.ActivationFunctionType
ALU = mybir.AluOpType


def _row_ap(src: bass.AP) -> bass.AP:
    """View a [N]-shaped DRAM AP as [1, N]."""
    return bass.AP(tensor=src.tensor, offset=src.offset, ap=[[0, 1]] + list(src.ap))


def build_fast_program() -> bass.Bass:
    """out = LN2(x @ W2c + fprow); W2c/fprow host-precomputed, bf16.

    x is DMA'd with an f32r dtype tag (same bits as f32) so the PE can
    transpose it directly at 1 cycle/col; the f32r->bf16 rounding happens
    for free inside the PSUM-evacuation copies, which are split between
    ACT and DVE to balance their per-tile loads.
    """
    nc = bacc.Bacc("TRN2", target_bir_lowering=False, debug=False)

    x = nc.dram_tensor("x", [ROWS, H], F32R, kind="ExternalInput").ap()
    w2c = nc.dram_tensor("w2c", [H, H], BF16, kind="ExternalInput").ap()   # [h,k]
    fpb = nc.dram_tensor("fpb", [H], BF16, kind="ExternalInput").ap()
    out = nc.dram_tensor("out", [ROWS, H], F32, kind="ExternalOutput").ap()

    with tile.TileContext(nc) as tc:
        with (
            tc.tile_pool(name="consts", bufs=1) as consts,
            tc.tile_pool(name="xs", bufs=8) as xs,
            tc.tile_pool(name="xcts", bufs=3) as xcts,
            tc.tile_pool(name="xctv", bufs=3) as xctv,
            tc.tile_pool(name="stats", bufs=6) as stats,
            tc.tile_pool(name="smalls", bufs=12) as smalls,
            tc.tile_pool(name="outs", bufs=3) as outs,
            tc.tile_pool(name="psum_t", bufs=3, space="PSUM") as psum_t,
            tc.tile_pool(name="psum_y", bufs=3, space="PSUM") as psum_y,
            tc.tile_pool(name="psum_d", bufs=1, space="PSUM") as psum_d,
        ):
            # ---- x loads: prefetch all 8 virtual tiles on the sync(SP)
            # ring.  Virtual tile v (c=v//2, j=v%2) holds DRAM rows
            # {256c + 2p + j}: one 2KB descriptor per partition at 4KB
            # stride, so a later store of tiles (2c, 2c+1) as [128,2,512]
            # needs only one contiguous 4KB descriptor per partition.
            x_all = []
            w2cs = consts.tile([P, 4, H], BF16)
            for v in range(NT):
                c, j = v // 2, v % 2
                xt = xs.tile([P, H], F32R)
                nc.sync.dma_start(
                    out=xt,
                    in_=x[c * 2 * P:(c + 1) * 2 * P, :].rearrange(
                        "(p two) k -> p two k", two=2)[:, j, :],
                )
                x_all.append(xt)

            # ---- one-time constants on the scalar(ACT) ring so they don't
            # delay x loads on the sync ring.  fp row (tiny) first: the ring
            # is FIFO and the first C-stage needs it.
            fpb_row = consts.tile([1, H], BF16)
            nc.scalar.dma_start(out=fpb_row, in_=_row_ap(fpb))
            ones_bf = consts.tile([1, P], BF16)
            nc.vector.memset(ones_bf, 1.0)

            # Identity build comes before the SWDGE weight loads: both run
            # on the Q7 and the first transposes are gated on the identity.
            iden_f32 = consts.tile([P, P], F32)
            make_identity(nc, iden_f32)
            iden = consts.tile([P, P], F32R)
            nc.gpsimd.tensor_copy(iden, iden_f32)

            # Per-chunk weight loads: chunk h must land before C-stage h
            # needs it.  Chunks 0/1 go through the GPSIMD SWDGE ring whose
            # descriptor generation runs on the otherwise-idle Q7, in
            # parallel with the HWDGE rings; 2/3 via the scalar ring.
            for h in range(4):
                eng = nc.gpsimd if h < 2 else nc.scalar
                eng.dma_start(out=w2cs[:, h, :], in_=w2c[h * P:(h + 1) * P, :])

            epst = consts.tile([P, 1], F32)
            nc.vector.memset(epst, EPS)

            # Hoist the ACT Sqrt table load into the preamble.
            dsq = smalls.tile([P, 1], F32, tag="dsq")
            nc.scalar.activation(dsq, epst, AF.Sqrt, bias=0.0, scale=1.0)

            # PE warm-up burst: keeps the tensor engine active while x0 is
            # in flight so HAM has up-clocked it before the real transposes.
            dwm = psum_d.tile([P, P], F32, tag="dwm")
            for _ in range(16):
                nc.tensor.matmul(dwm, ones_bf, ones_bf, start=True, stop=True)
            # Dummy PE op absorbing the identity's GPSIMD wait.
            d1 = psum_d.tile([P, P], F32R, tag="dummy")
            nc.tensor.transpose(d1, iden, iden)

            # ---- software-pipelined main loop over 8 token tiles ----
            # B(t): PE transposes tile t; evac adds fp chunk bias and casts
            #       to bf16 (2 chunks on ACT, 2 on DVE)
            # C(t-1): W2c matmuls + LN2 stats/apply
            # store chunk c after C(2c+1), interleaved like the loads.
            xct_all, ot_ch = {}, {}
            for i in range(NT + 1):
                if i < NT:
                    t = i
                    xt = x_all[t]
                    ptr = psum_t.tile([P, 4, P], F32R)
                    for h in range(4):
                        nc.tensor.transpose(ptr[:, h, :], xt[:, h * P:(h + 1) * P],
                                            iden)
                    xct_a = xcts.tile([P, 2, P], BF16)
                    nc.scalar.copy(xct_a, ptr[:, 0:2, :])
                    xct_b = xctv.tile([P, 2, P], BF16)
                    nc.vector.tensor_copy(xct_b, ptr[:, 2:4, :])
                    xct_all[t] = (xct_a, xct_b)

                if i >= 1:
                    k = i - 1
                    xct_a, xct_b = xct_all[k]
                    py = psum_y.tile([P, H], F32)
                    nc.tensor.matmul(py, ones_bf, fpb_row, start=True, stop=False)
                    for h in range(4):
                        src = xct_a[:, h, :] if h < 2 else xct_b[:, h - 2, :]
                        nc.tensor.matmul(
                            py, src, w2cs[:, h, :],
                            start=False, stop=(h == 3),
                        )

                    st2 = stats.tile([P, 6], F32, tag="st")
                    nc.vector.bn_stats(st2, py)
                    mv2 = stats.tile([P, 2], F32, tag="mv")
                    nc.vector.bn_aggr(mv2, st2)
                    sd2 = smalls.tile([P, 1], F32, tag="sd")
                    nc.scalar.activation(sd2, mv2[:, 1:2], AF.Sqrt, bias=epst,
                                         scale=1.0)
                    s2 = smalls.tile([P, 1], F32, tag="s")
                    nc.vector.reciprocal(s2, sd2)
                    negms2 = smalls.tile([P, 1], F32, tag="negms")
                    # Last two tiles: keep the chain on DVE — the GPSIMD hop
                    # adds ~300ns of latency to the pipeline drain.
                    neg_eng = nc.vector if k >= NT - 2 else nc.gpsimd
                    neg_eng.tensor_scalar(
                        negms2, mv2[:, 0:1], s2, -1.0, op0=ALU.mult, op1=ALU.mult
                    )

                    c, j = k // 2, k % 2
                    if j == 0:
                        ot2 = outs.tile([P, 2, H], F32)
                        ot_ch[c] = ot2
                    nc.scalar.activation(ot_ch[c][:, j, :], py, AF.Identity,
                                         bias=negms2, scale=s2)
                    if j == 1:
                        nc.sync.dma_start(
                            out=out[c * 2 * P:(c + 1) * 2 * P, :].rearrange(
                                "(p two) k -> p two k", two=2),
                            in_=ot_ch[c],
                        )

    nc.compile()
    return nc


# ---------------------------------------------------------------------------
# General (slow, fully-correct) program: original pipeline with LN1.
# Used only when c2 != 0 or g2 != 1 or b2 != 0.
# ---------------------------------------------------------------------------

def _bcast_ap(src: bass.AP, parts: int) -> bass.AP:
    return bass.AP(tensor=src.tensor, offset=src.offset, ap=[[0, parts]] + list(src.ap))


def build_general_program() -> bass.Bass:
    nc = bacc.Bacc("TRN2", target_bir_lowering=False, debug=False)

    x = nc.dram_tensor("x", [ROWS, H], F32, kind="ExternalInput").ap()
    w2 = nc.dram_tensor("w2", [H, H], F32, kind="ExternalInput").ap()   # [h,k]
    c2 = nc.dram_tensor("c2", [H], F32, kind="ExternalInput").ap()
    fp = nc.dram_tensor("fp", [H], F32, kind="ExternalInput").ap()
    g2 = nc.dram_tensor("g2", [H], F32, kind="ExternalInput").ap()
    b2 = nc.dram_tensor("b2", [H], F32, kind="ExternalInput").ap()
    out = nc.dram_tensor("out", [ROWS, H], F32, kind="ExternalOutput").ap()

    MD = F32R

    with tile.TileContext(nc) as tc:
        with (
            tc.tile_pool(name="consts", bufs=1) as consts,
            tc.tile_pool(name="xs", bufs=4) as xs,
            tc.tile_pool(name="zs", bufs=4) as zs,
            tc.tile_pool(name="xns", bufs=8) as xns,
            tc.tile_pool(name="xnts", bufs=3) as xnts,
            tc.tile_pool(name="stats", bufs=6) as stats,
            tc.tile_pool(name="smalls", bufs=12) as smalls,
            tc.tile_pool(name="ts", bufs=3) as ts_pool,
            tc.tile_pool(name="outs", bufs=3) as outs,
            tc.tile_pool(name="psum_t", bufs=3, space="PSUM") as psum_t,
            tc.tile_pool(name="psum_y", bufs=3, space="PSUM") as psum_y,
            tc.tile_pool(name="psum_d", bufs=1, space="PSUM") as psum_d,
        ):
            ones1 = consts.tile([1, P], F32)
            nc.vector.memset(ones1, 1.0)
            onesmm = consts.tile([1, P], MD)
            nc.vector.tensor_copy(onesmm, ones1)

            fprow = consts.tile([1, H], F32)
            nc.sync.dma_start(out=fprow, in_=_row_ap(fp))
            fpmm = consts.tile([1, H], MD)
            nc.vector.tensor_copy(fpmm, fprow)
            fp_ps = psum_d.tile([P, H], F32, tag="bcast")
            nc.tensor.matmul(fp_ps, onesmm, fpmm, start=True, stop=True)
            fpb = consts.tile([P, H], F32)
            nc.scalar.copy(fpb, fp_ps)

            g2b = consts.tile([P, H], F32)
            nc.gpsimd.dma_start(out=g2b, in_=_bcast_ap(g2, P))
            b2b = consts.tile([P, H], F32)
            nc.gpsimd.dma_start(out=b2b, in_=_bcast_ap(b2, P))

            c2row = consts.tile([1, H], F32)
            nc.sync.dma_start(out=c2row, in_=_row_ap(c2))
            c2mm = consts.tile([1, H], MD)
            nc.vector.tensor_copy(c2mm, c2row)

            iden_f32 = consts.tile([P, P], F32)
            make_identity(nc, iden_f32)
            iden = consts.tile([P, P], F32R)
            nc.gpsimd.tensor_copy(iden, iden_f32)
            epst = consts.tile([P, 1], F32)
            nc.vector.memset(epst, EPS)

            d1 = psum_d.tile([P, P], MD, tag="dummy")
            nc.tensor.transpose(d1, iden, iden)

            xn_all, xnt_all = {}, {}
            w2mm = consts.tile([P, 4, H], MD)
            for i in range(NT + 3):
                if i == 1:
                    w2s = consts.tile([P, 4, H], F32)
                    nc.sync.dma_start(
                        out=w2s, in_=w2.rearrange("(t p) k -> p t k", p=P)
                    )
                    nc.scalar.copy(w2mm, w2s)

                if i < NT:
                    xt = xs.tile([P, H], F32)
                    nc.sync.dma_start(out=xt, in_=x[i * P:(i + 1) * P, :])

                    z = zs.tile([P, H], F32)
                    nc.vector.tensor_add(z, xt, fpb)

                    st1 = stats.tile([P, 6], F32, tag="st")
                    nc.vector.bn_stats(st1, z)
                    mv1 = stats.tile([P, 2], F32, tag="mv")
                    nc.vector.bn_aggr(mv1, st1)
                    sd1 = smalls.tile([P, 1], F32, tag="sd")
                    nc.scalar.activation(sd1, mv1[:, 1:2], AF.Sqrt, bias=epst,
                                         scale=1.0)
                    s1 = smalls.tile([P, 1], F32, tag="s")
                    nc.vector.reciprocal(s1, sd1)
                    negms1 = smalls.tile([P, 1], F32, tag="negms")
                    nc.vector.tensor_scalar(
                        negms1, mv1[:, 0:1], s1, -1.0, op0=ALU.mult, op1=ALU.mult
                    )
                    xn = xns.tile([P, H], MD)
                    nc.scalar.activation(xn, z, AF.Identity, bias=negms1, scale=s1)
                    xn_all[i] = xn

                if 2 <= i < NT + 2:
                    j = i - 2
                    xn = xn_all[j]
                    ptr = psum_t.tile([P, 4, P], MD)
                    for h in range(4):
                        nc.tensor.transpose(ptr[:, h, :], xn[:, h * P:(h + 1) * P],
                                            iden)
                    xnt = xnts.tile([P, 4, P], MD)
                    nc.scalar.copy(xnt, ptr)
                    xnt_all[j] = xnt

                if i >= 3:
                    k = i - 3
                    xnt = xnt_all[k]
                    py = psum_y.tile([P, H], F32)
                    nc.tensor.matmul(py, onesmm, c2mm, start=True, stop=False)
                    for h in range(4):
                        nc.tensor.matmul(
                            py, xnt[:, h, :], w2mm[:, h, :],
                            start=False, stop=(h == 3),
                        )

                    st2 = stats.tile([P, 6], F32, tag="st")
                    nc.vector.bn_stats(st2, py)
                    mv2 = stats.tile([P, 2], F32, tag="mv")
                    nc.vector.bn_aggr(mv2, st2)
                    sd2 = smalls.tile([P, 1], F32, tag="sd")
                    nc.scalar.activation(sd2, mv2[:, 1:2], AF.Sqrt, bias=epst,
                                         scale=1.0)
                    s2 = smalls.tile([P, 1], F32, tag="s")
                    nc.vector.reciprocal(s2, sd2)
                    negms2 = smalls.tile([P, 1], F32, tag="negms")
                    nc.vector.tensor_scalar(
                        negms2, mv2[:, 0:1], s2, -1.0, op0=ALU.mult, op1=ALU.mult
                    )

                    t = ts_pool.tile([P, H], F32)
                    nc.scalar.activation(t, py, AF.Identity, bias=negms2, scale=s2)

                    t2 = outs.tile([P, H], F32, tag="t2")
                    nc.gpsimd.tensor_mul(t2, t, g2b)
                    ot = outs.tile([P, H], F32, tag="ot")
                    nc.gpsimd.tensor_add(ot, t2, b2b)

                    nc.sync.dma_start(out=out[k * P:(k + 1) * P, :], in_=ot)

    nc.compile()
    return nc


def _weights(Wo, g1, b1, bo):
    f32 = np.float32
    W2 = g1[:, None] * (Wo.T + np.eye(H, dtype=f32))           # [h,k]
    c2 = b1 + bo + Wo @ b1                                     # [k]
    return W2, c2


def _host_prep(temporal_features, static_features, Wt, bt, Wf, bf, Wo, bo,
               g1, b1, g2, b2):
    f32 = np.float32
    x = np.ascontiguousarray(np.asarray(temporal_features, dtype=f32)).reshape(B * S, H)
    st = np.asarray(static_features, dtype=f32)
    Wf = np.asarray(Wf, dtype=f32)
    bf = np.asarray(bf, dtype=f32)
    Wo = np.asarray(Wo, dtype=f32)
    bo = np.asarray(bo, dtype=f32)
    g1 = np.asarray(g1, dtype=f32)
    b1 = np.asarray(b1, dtype=f32)
    g2 = np.asarray(g2, dtype=f32)
    b2 = np.asarray(b2, dtype=f32)

    fp = st @ Wf.T + bf                                        # [B,H]
    W2, c2 = _weights(Wo, g1, b1, bo)

    fast = (not np.any(c2 != 0.0)) and (not np.any(g2 != 1.0)) \
        and (not np.any(b2 != 0.0))

    in_maps = []
    if fast:
        import ml_dtypes
        bf16 = np.dtype(ml_dtypes.bfloat16)
        wcs = W2.sum(axis=0)                                   # [k]
        W2c = W2 - wcs[None, :] / f32(H)                       # centered cols
        w2c_bf = np.ascontiguousarray(W2c.astype(bf16))
        fpw2c = (fp @ W2c).astype(f32)                          # [B,H]
        for c in range(N_CORES):
            shard = np.ascontiguousarray(x[c * ROWS:(c + 1) * ROWS])
            row = fpw2c[(c * ROWS) // S]
            in_maps.append({
                "x": shard,
                "w2c": w2c_bf,
                "fpb": np.ascontiguousarray(row.astype(bf16)),
            })
        return True, in_maps

    for c in range(N_CORES):
        shard = np.ascontiguousarray(x[c * ROWS:(c + 1) * ROWS])
        in_maps.append({
            "x": shard,
            "w2": np.ascontiguousarray(W2),
            "c2": np.ascontiguousarray(c2),
            "fp": np.ascontiguousarray(fp[(c * ROWS) // S]),
            "g2": np.ascontiguousarray(g2),
            "b2": np.ascontiguousarray(b2),
        })
    return False, in_maps


_NC_CACHE = {}


def _get_program(fast: bool):
    if fast not in _NC_CACHE:
        _NC_CACHE[fast] = build_fast_program() if fast else build_general_program()
    return _NC_CACHE[fast]


def run(inputs: dict, trace: bool = False):
    """Returns (output [B,S,H] f32, BassKernelResults)."""
    fast, in_maps = _host_prep(**inputs)
    nc = _get_program(fast)
    res = run_bass_kernel_spmd(nc, in_maps, list(range(N_CORES)), trace=trace)
    shards = [res.results[c]["out"] for c in range(N_CORES)]
    full = np.concatenate(shards, axis=0).reshape(B, S, H).astype(np.float32)
    return full, res


def kernel(**inputs) -> np.ndarray:
    out, _ = run(inputs, trace=False)
    return out


# revision 47
# speedup vs baseline: 1.0605x; 1.0014x over previous
"""Trainium2 Bass kernel for nn_CrossDimensionalAttention_60550448939365.

Math reduction (see reference): scores[b,i,j] = tp[b,i] . fp[b] is constant in
j, so softmax over j is exactly uniform and attended[b,i,:] = fp[b,:].  The
whole Wt/scores/softmax/bmm pipeline is a no-op.  What remains:

    z   = x + fp[b]                 (broadcast over seq)
    y   = LN1(z) @ W2 + c2          W2 = g1[:,None]*(Wo.T + I)
                                    c2 = b1 + bo + Wo @ b1
    out = LN2(y) * g2 + b2

Fast path (c2 == 0, g2 == 1, b2 == 0 -- true for this checkpoint):
LN1's 1/sd row scale cancels inside LN2 (scale invariance), and the row-mean
centering folds into the weights:

    (z - rowmean(z)) @ W2 = z @ W2c,   W2c = W2 - ones * colsum(W2)/H
    out = LN2(x @ W2c + fp @ W2c)

so the device does NO LN1 work at all: cast x to bf16, transpose via PE,
matmul with host-precast bf16 W2c (plus two rank-1 terms adding the constant
fp@W2c row in hi+lo bf16 halves), then LN2 stats + apply, store.  A general
program (the original full pipeline) is built when the fast-path conditions
don't hold, so kernel() is correct for any inputs.

Sharding: rows of flattened [B*S, H] = [8192, 512] split evenly across the 8
cores (1024 rows each, each shard entirely within one batch b = core//2).
"""

import os
import numpy as np

import concourse.bass as bass
import concourse.tile as tile
from concourse import bacc, mybir
from concourse.bass_utils import run_bass_kernel_spmd
from concourse.masks import make_identity

H = 512
B = 4
S = 2048
N_CORES = 8
ROWS = (B * S) // N_CORES  # 1024 rows per core
P = 128
NT = ROWS // P             # 8 token tiles per core
EPS = 1e-5

F32 = mybir.dt.float32
F32R = mybir.dt.float32r
BF16 = mybir.dt.bfloat16
AF = mybir.ActivationFunctionType
ALU = mybir.AluOpType


def _row_ap(src: bass.AP) -> bass.AP:
    """View a [N]-shaped DRAM AP as [1, N]."""
    return bass.AP(tensor=src.tensor, offset=src.offset, ap=[[0, 1]] + list(src.ap))


def build_fast_program() -> bass.Bass:
    """out = LN2(x @ W2c + fprow); W2c/fprow host-precomputed, bf16.

    x is DMA'd with an f32r dtype tag (same bits as f32) so the PE can
    transpose it directly at 1 cycle/col; the f32r->bf16 rounding happens
    for free inside the PSUM-evacuation copies, which are split between
    ACT and DVE to balance their per-tile loads.
    """
    nc = bacc.Bacc("TRN2", target_bir_lowering=False, debug=False)

    x = nc.dram_tensor("x", [ROWS, H], F32R, kind="ExternalInput").ap()
    w2c = nc.dram_tensor("w2c", [H, H], BF16, kind="ExternalInput").ap()   # [h,k]
    fpb = nc.dram_tensor("fpb", [H], BF16, kind="ExternalInput").ap()
    out = nc.dram_tensor("out", [ROWS, H], F32, kind="ExternalOutput").ap()

    with tile.TileContext(nc) as tc:
        with (
            tc.tile_pool(name="consts", bufs=1) as consts,
            tc.tile_pool(name="xs", bufs=8) as xs,
            tc.tile_pool(name="xcts", bufs=3) as xcts,
            tc.tile_pool(name="xctv", bufs=3) as xctv,
            tc.tile_pool(name="stats", bufs=6) as stats,
            tc.tile_pool(name="smalls", bufs=12) as smalls,
            tc.tile_pool(name="outs", bufs=3) as outs,
            tc.tile_pool(name="psum_t", bufs=3, space="PSUM") as psum_t,
            tc.tile_pool(name="psum_y", bufs=3, space="PSUM") as psum_y,
            tc.tile_pool(name="psum_d", bufs=1, space="PSUM") as psum_d,
        ):
            # ---- x loads: prefetch all 8 virtual tiles on the sync(SP)
            # ring.  Virtual tile v (c=v//2, j=v%2) holds DRAM rows
            # {256c + 2p + j}: one 2KB descriptor per partition at 4KB
            # stride, so a later store of tiles (2c, 2c+1) as [128,2,512]
            # needs only one contiguous 4KB descriptor per partition.
            x_all = []
            w2cs = consts.tile([P, 4, H], BF16)
            for v in range(NT):
                c, j = v // 2, v % 2
                xt = xs.tile([P, H], F32R)
                nc.sync.dma_start(
                    out=xt,
                    in_=x[c * 2 * P:(c + 1) * 2 * P, :].rearrange(
                        "(p two) k -> p two k", two=2)[:, j, :],
                )
                x_all.append(xt)

            # ---- one-time constants on the scalar(ACT) ring so they don't
            # delay x loads on the sync ring.  fp row (tiny) first: the ring
            # is FIFO and the first C-stage needs it.
            fpb_row = consts.tile([1, H], BF16)
            nc.scalar.dma_start(out=fpb_row, in_=_row_ap(fpb))
            ones_bf = consts.tile([1, P], BF16)
            nc.vector.memset(ones_bf, 1.0)

            # Identity build comes before the SWDGE weight loads: both run
            # on the Q7 and the first transposes are gated on the identity.
            iden_f32 = consts.tile([P, P], F32)
            make_identity(nc, iden_f32)
            iden = consts.tile([P, P], F32R)
            nc.gpsimd.tensor_copy(iden, iden_f32)

            # Per-chunk weight loads: chunk h must land before C-stage h
            # needs it.  Chunks 0/1 go through the GPSIMD SWDGE ring whose
            # descriptor generation runs on the otherwise-idle Q7, in
            # parallel with the HWDGE rings; 2/3 via the scalar ring.
            for h in range(4):
                eng = nc.gpsimd if h < 2 else nc.scalar
                eng.dma_start(out=w2cs[:, h, :], in_=w2c[h * P:(h + 1) * P, :])

            epst = consts.tile([P, 1], F32)
            nc.vector.memset(epst, EPS)

            # Hoist the ACT Sqrt table load into the preamble.
            dsq = smalls.tile([P, 1], F32, tag="dsq")
            nc.scalar.activation(dsq, epst, AF.Sqrt, bias=0.0, scale=1.0)

            # PE warm-up burst: keeps the tensor engine active while x0 is
            # in flight so HAM has up-clocked it before the real transposes.
            dwm = psum_d.tile([P, P], F32, tag="dwm")
            for _ in range(16):
                nc.tensor.matmul(dwm, ones_bf, ones_bf, start=True, stop=True)
            # Dummy PE op absorbing the identity's GPSIMD wait.
            d1 = psum_d.tile([P, P], F32R, tag="dummy")
            nc.tensor.transpose(d1, iden, iden)

            # ---- software-pipelined main loop over 8 token tiles ----
            # B(t): PE transposes tile t; evac adds fp chunk bias and casts
            #       to bf16 (2 chunks on ACT, 2 on DVE)
            # C(t-1): W2c matmuls + LN2 stats/apply
            # store chunk c after C(2c+1), interleaved like the loads.
            xct_all, ot_ch = {}, {}
            for i in range(NT + 1):
                if i < NT:
                    t = i
                    xt = x_all[t]
                    ptr = psum_t.tile([P, 4, P], F32R)
                    for h in range(4):
                        nc.tensor.transpose(ptr[:, h, :], xt[:, h * P:(h + 1) * P],
                                            iden)
                    xct_a = xcts.tile([P, 2, P], BF16)
                    nc.scalar.copy(xct_a, ptr[:, 0:2, :])
                    xct_b = xctv.tile([P, 2, P], BF16)
                    nc.vector.tensor_copy(xct_b, ptr[:, 2:4, :])
                    xct_all[t] = (xct_a, xct_b)

                if i >= 1:
                    k = i - 1
                    xct_a, xct_b = xct_all[k]
                    py = psum_y.tile([P, H], F32)
                    nc.tensor.matmul(py, ones_bf, fpb_row, start=True, stop=False)
                    for h in range(4):
                        src = xct_a[:, h, :] if h < 2 else xct_b[:, h - 2, :]
                        nc.tensor.matmul(
                            py, src, w2cs[:, h, :],
                            start=False, stop=(h == 3),
                        )

                    st2 = stats.tile([P, 6], F32, tag="st")
                    nc.vector.bn_stats(st2, py)
                    mv2 = stats.tile([P, 2], F32, tag="mv")
                    nc.vector.bn_aggr(mv2, st2)
                    sd2 = smalls.tile([P, 1], F32, tag="sd")
                    nc.scalar.activation(sd2, mv2[:, 1:2], AF.Sqrt, bias=epst,
                                         scale=1.0)
                    s2 = smalls.tile([P, 1], F32, tag="s")
                    nc.vector.reciprocal(s2, sd2)
                    negms2 = smalls.tile([P, 1], F32, tag="negms")
                    # Last two tiles: keep the chain on DVE — the GPSIMD hop
                    # adds ~300ns of latency to the pipeline drain.
                    neg_eng = nc.vector if k >= NT - 2 else nc.gpsimd
                    neg_eng.tensor_scalar(
                        negms2, mv2[:, 0:1], s2, -1.0, op0=ALU.mult, op1=ALU.mult
                    )

                    c, j = k // 2, k % 2
                    out_view = out[c * 2 * P:(c + 1) * 2 * P, :].rearrange(
                        "(p two) k -> p two k", two=2)
                    if k >= NT - 2:
                        # Last chunk: store per tile so tile 6's store
                        # overlaps tile 7's LN2 chain and the final transfer
                        # is half the size -- this store is the kernel's
                        # critical-path exit.
                        ots = outs.tile([P, H], F32, tag="single")
                        nc.scalar.activation(ots, py, AF.Identity,
                                             bias=negms2, scale=s2)
                        nc.sync.dma_start(out=out_view[:, j, :], in_=ots)
                    else:
                        if j == 0:
                            ot2 = outs.tile([P, 2, H], F32)
                            ot_ch[c] = ot2
                        nc.scalar.activation(ot_ch[c][:, j, :], py, AF.Identity,
                                             bias=negms2, scale=s2)
                        if j == 1:
                            nc.sync.dma_start(out=out_view, in_=ot_ch[c])

    nc.compile()
    return nc


# ---------------------------------------------------------------------------
# General (slow, fully-correct) program: original pipeline with LN1.
# Used only when c2 != 0 or g2 != 1 or b2 != 0.
# ---------------------------------------------------------------------------

def _bcast_ap(src: bass.AP, parts: int) -> bass.AP:
    return bass.AP(tensor=src.tensor, offset=src.offset, ap=[[0, parts]] + list(src.ap))


def build_general_program() -> bass.Bass:
    nc = bacc.Bacc("TRN2", target_bir_lowering=False, debug=False)

    x = nc.dram_tensor("x", [ROWS, H], F32, kind="ExternalInput").ap()
    w2 = nc.dram_tensor("w2", [H, H], F32, kind="ExternalInput").ap()   # [h,k]
    c2 = nc.dram_tensor("c2", [H], F32, kind="ExternalInput").ap()
    fp = nc.dram_tensor("fp", [H], F32, kind="ExternalInput").ap()
    g2 = nc.dram_tensor("g2", [H], F32, kind="ExternalInput").ap()
    b2 = nc.dram_tensor("b2", [H], F32, kind="ExternalInput").ap()
    out = nc.dram_tensor("out", [ROWS, H], F32, kind="ExternalOutput").ap()

    MD = F32R

    with tile.TileContext(nc) as tc:
        with (
            tc.tile_pool(name="consts", bufs=1) as consts,
            tc.tile_pool(name="xs", bufs=4) as xs,
            tc.tile_pool(name="zs", bufs=4) as zs,
            tc.tile_pool(name="xns", bufs=8) as xns,
            tc.tile_pool(name="xnts", bufs=3) as xnts,
            tc.tile_pool(name="stats", bufs=6) as stats,
            tc.tile_pool(name="smalls", bufs=12) as smalls,
            tc.tile_pool(name="ts", bufs=3) as ts_pool,
            tc.tile_pool(name="outs", bufs=3) as outs,
            tc.tile_pool(name="psum_t", bufs=3, space="PSUM") as psum_t,
            tc.tile_pool(name="psum_y", bufs=3, space="PSUM") as psum_y,
            tc.tile_pool(name="psum_d", bufs=1, space="PSUM") as psum_d,
        ):
            ones1 = consts.tile([1, P], F32)
            nc.vector.memset(ones1, 1.0)
            onesmm = consts.tile([1, P], MD)
            nc.vector.tensor_copy(onesmm, ones1)

            fprow = consts.tile([1, H], F32)
            nc.sync.dma_start(out=fprow, in_=_row_ap(fp))
            fpmm = consts.tile([1, H], MD)
            nc.vector.tensor_copy(fpmm, fprow)
            fp_ps = psum_d.tile([P, H], F32, tag="bcast")
            nc.tensor.matmul(fp_ps, onesmm, fpmm, start=True, stop=True)
            fpb = consts.tile([P, H], F32)
            nc.scalar.copy(fpb, fp_ps)

            g2b = consts.tile([P, H], F32)
            nc.gpsimd.dma_start(out=g2b, in_=_bcast_ap(g2, P))
            b2b = consts.tile([P, H], F32)
            nc.gpsimd.dma_start(out=b2b, in_=_bcast_ap(b2, P))

            c2row = consts.tile([1, H], F32)
            nc.sync.dma_start(out=c2row, in_=_row_ap(c2))
            c2mm = consts.tile([1, H], MD)
            nc.vector.tensor_copy(c2mm, c2row)

            iden_f32 = consts.tile([P, P], F32)
            make_identity(nc, iden_f32)
            iden = consts.tile([P, P], F32R)
            nc.gpsimd.tensor_copy(iden, iden_f32)
            epst = consts.tile([P, 1], F32)
            nc.vector.memset(epst, EPS)

            d1 = psum_d.tile([P, P], MD, tag="dummy")
            nc.tensor.transpose(d1, iden, iden)

            xn_all, xnt_all = {}, {}
            w2mm = consts.tile([P, 4, H], MD)
            for i in range(NT + 3):
                if i == 1:
                    w2s = consts.tile([P, 4, H], F32)
                    nc.sync.dma_start(
                        out=w2s, in_=w2.rearrange("(t p) k -> p t k", p=P)
                    )
                    nc.scalar.copy(w2mm, w2s)

                if i < NT:
                    xt = xs.tile([P, H], F32)
                    nc.sync.dma_start(out=xt, in_=x[i * P:(i + 1) * P, :])

                    z = zs.tile([P, H], F32)
                    nc.vector.tensor_add(z, xt, fpb)

                    st1 = stats.tile([P, 6], F32, tag="st")
                    nc.vector.bn_stats(st1, z)
                    mv1 = stats.tile([P, 2], F32, tag="mv")
                    nc.vector.bn_aggr(mv1, st1)
                    sd1 = smalls.tile([P, 1], F32, tag="sd")
                    nc.scalar.activation(sd1, mv1[:, 1:2], AF.Sqrt, bias=epst,
                                         scale=1.0)
                    s1 = smalls.tile([P, 1], F32, tag="s")
                    nc.vector.reciprocal(s1, sd1)
                    negms1 = smalls.tile([P, 1], F32, tag="negms")
                    nc.vector.tensor_scalar(
                        negms1, mv1[:, 0:1], s1, -1.0, op0=ALU.mult, op1=ALU.mult
                    )
                    xn = xns.tile([P, H], MD)
                    nc.scalar.activation(xn, z, AF.Identity, bias=negms1, scale=s1)
                    xn_all[i] = xn

                if 2 <= i < NT + 2:
                    j = i - 2
                    xn = xn_all[j]
                    ptr = psum_t.tile([P, 4, P], MD)
                    for h in range(4):
                        nc.tensor.transpose(ptr[:, h, :], xn[:, h * P:(h + 1) * P],
                                            iden)
                    xnt = xnts.tile([P, 4, P], MD)
                    nc.scalar.copy(xnt, ptr)
                    xnt_all[j] = xnt

                if i >= 3:
                    k = i - 3
                    xnt = xnt_all[k]
                    py = psum_y.tile([P, H], F32)
                    nc.tensor.matmul(py, onesmm, c2mm, start=True, stop=False)
                    for h in range(4):
                        nc.tensor.matmul(
                            py, xnt[:, h, :], w2mm[:, h, :],
                            start=False, stop=(h == 3),
                        )

                    st2 = stats.tile([P, 6], F32, tag="st")
                    nc.vector.bn_stats(st2, py)
                    mv2 = stats.tile([P, 2], F32, tag="mv")
                    nc.vector.bn_aggr(mv2, st2)
                    sd2 = smalls.tile([P, 1], F32, tag="sd")
                    nc.scalar.activation(sd2, mv2[:, 1:2], AF.Sqrt, bias=epst,
                                         scale=1.0)
                    s2 = smalls.tile([P, 1], F32, tag="s")
                    nc.vector.reciprocal(s2, sd2)
                    negms2 = smalls.tile([P, 1], F32, tag="negms")
                    nc.vector.tensor_scalar(
                        negms2, mv2[:, 0:1], s2, -1.0, op0=ALU.mult, op1=ALU.mult
                    )

                    t = ts_pool.tile([P, H], F32)
                    nc.scalar.activation(t, py, AF.Identity, bias=negms2, scale=s2)

                    t2 = outs.tile([P, H], F32, tag="t2")
                    nc.gpsimd.tensor_mul(t2, t, g2b)
                    ot = outs.tile([P, H], F32, tag="ot")
                    nc.gpsimd.tensor_add(ot, t2, b2b)

                    nc.sync.dma_start(out=out[k * P:(k + 1) * P, :], in_=ot)

    nc.compile()
    return nc


def _weights(Wo, g1, b1, bo):
    f32 = np.float32
    W2 = g1[:, None] * (Wo.T + np.eye(H, dtype=f32))           # [h,k]
    c2 = b1 + bo + Wo @ b1                                     # [k]
    return W2, c2


def _host_prep(temporal_features, static_features, Wt, bt, Wf, bf, Wo, bo,
               g1, b1, g2, b2):
    f32 = np.float32
    x = np.ascontiguousarray(np.asarray(temporal_features, dtype=f32)).reshape(B * S, H)
    st = np.asarray(static_features, dtype=f32)
    Wf = np.asarray(Wf, dtype=f32)
    bf = np.asarray(bf, dtype=f32)
    Wo = np.asarray(Wo, dtype=f32)
    bo = np.asarray(bo, dtype=f32)
    g1 = np.asarray(g1, dtype=f32)
    b1 = np.asarray(b1, dtype=f32)
    g2 = np.asarray(g2, dtype=f32)
    b2 = np.asarray(b2, dtype=f32)

    fp = st @ Wf.T + bf                                        # [B,H]
    W2, c2 = _weights(Wo, g1, b1, bo)

    fast = (not np.any(c2 != 0.0)) and (not np.any(g2 != 1.0)) \
        and (not np.any(b2 != 0.0))

    in_maps = []
    if fast:
        import ml_dtypes
        bf16 = np.dtype(ml_dtypes.bfloat16)
        wcs = W2.sum(axis=0)                                   # [k]
        W2c = W2 - wcs[None, :] / f32(H)                       # centered cols
        w2c_bf = np.ascontiguousarray(W2c.astype(bf16))
        fpw2c = (fp @ W2c).astype(f32)                          # [B,H]
        for c in range(N_CORES):
            shard = np.ascontiguousarray(x[c * ROWS:(c + 1) * ROWS])
            row = fpw2c[(c * ROWS) // S]
            in_maps.append({
                "x": shard,
                "w2c": w2c_bf,
                "fpb": np.ascontiguousarray(row.astype(bf16)),
            })
        return True, in_maps

    for c in range(N_CORES):
        shard = np.ascontiguousarray(x[c * ROWS:(c + 1) * ROWS])
        in_maps.append({
            "x": shard,
            "w2": np.ascontiguousarray(W2),
            "c2": np.ascontiguousarray(c2),
            "fp": np.ascontiguousarray(fp[(c * ROWS) // S]),
            "g2": np.ascontiguousarray(g2),
            "b2": np.ascontiguousarray(b2),
        })
    return False, in_maps


_NC_CACHE = {}


def _get_program(fast: bool):
    if fast not in _NC_CACHE:
        _NC_CACHE[fast] = build_fast_program() if fast else build_general_program()
    return _NC_CACHE[fast]


def run(inputs: dict, trace: bool = False):
    """Returns (output [B,S,H] f32, BassKernelResults)."""
    fast, in_maps = _host_prep(**inputs)
    nc = _get_program(fast)
    res = run_bass_kernel_spmd(nc, in_maps, list(range(N_CORES)), trace=trace)
    shards = [res.results[c]["out"] for c in range(N_CORES)]
    full = np.concatenate(shards, axis=0).reshape(B, S, H).astype(np.float32)
    return full, res


def kernel(**inputs) -> np.ndarray:
    out, _ = run(inputs, trace=False)
    return out


# revision 48
# speedup vs baseline: 1.0846x; 1.0227x over previous
"""Trainium2 Bass kernel for nn_CrossDimensionalAttention_60550448939365.

Math reduction (see reference): scores[b,i,j] = tp[b,i] . fp[b] is constant in
j, so softmax over j is exactly uniform and attended[b,i,:] = fp[b,:].  The
whole Wt/scores/softmax/bmm pipeline is a no-op.  What remains:

    z   = x + fp[b]                 (broadcast over seq)
    y   = LN1(z) @ W2 + c2          W2 = g1[:,None]*(Wo.T + I)
                                    c2 = b1 + bo + Wo @ b1
    out = LN2(y) * g2 + b2

Fast path (c2 == 0, g2 == 1, b2 == 0 -- true for this checkpoint):
LN1's 1/sd row scale cancels inside LN2 (scale invariance), and the row-mean
centering folds into the weights:

    (z - rowmean(z)) @ W2 = z @ W2c,   W2c = W2 - ones * colsum(W2)/H
    out = LN2(x @ W2c + fp @ W2c)

so the device does NO LN1 work at all: cast x to bf16, transpose via PE,
matmul with host-precast bf16 W2c (plus two rank-1 terms adding the constant
fp@W2c row in hi+lo bf16 halves), then LN2 stats + apply, store.  A general
program (the original full pipeline) is built when the fast-path conditions
don't hold, so kernel() is correct for any inputs.

Sharding: rows of flattened [B*S, H] = [8192, 512] split evenly across the 8
cores (1024 rows each, each shard entirely within one batch b = core//2).
"""

import os
import numpy as np

import concourse.bass as bass
import concourse.tile as tile
from concourse import bacc, mybir
from concourse.bass_utils import run_bass_kernel_spmd
from concourse.masks import make_identity

H = 512
B = 4
S = 2048
N_CORES = 8
ROWS = (B * S) // N_CORES  # 1024 rows per core
P = 128
NT = ROWS // P             # 8 token tiles per core
EPS = 1e-5

F32 = mybir.dt.float32
F32R = mybir.dt.float32r
BF16 = mybir.dt.bfloat16
AF = mybir.ActivationFunctionType
ALU = mybir.AluOpType


def _row_ap(src: bass.AP) -> bass.AP:
    """View a [N]-shaped DRAM AP as [1, N]."""
    return bass.AP(tensor=src.tensor, offset=src.offset, ap=[[0, 1]] + list(src.ap))


def build_fast_program() -> bass.Bass:
    """out = LN2(x @ W2c + fprow); W2c/fprow host-precomputed, bf16.

    x is DMA'd with an f32r dtype tag (same bits as f32) so the PE can
    transpose it directly at 1 cycle/col; the f32r->bf16 rounding happens
    for free inside the PSUM-evacuation copies, which are split between
    ACT and DVE to balance their per-tile loads.
    """
    nc = bacc.Bacc("TRN2", target_bir_lowering=False, debug=False)

    x = nc.dram_tensor("x", [ROWS, H], F32R, kind="ExternalInput").ap()
    w2c = nc.dram_tensor("w2c", [H, H], BF16, kind="ExternalInput").ap()   # [h,k]
    fpb = nc.dram_tensor("fpb", [H], BF16, kind="ExternalInput").ap()
    out = nc.dram_tensor("out", [ROWS, H], F32, kind="ExternalOutput").ap()

    with tile.TileContext(nc) as tc:
        with (
            tc.tile_pool(name="consts", bufs=1) as consts,
            tc.tile_pool(name="xs", bufs=8) as xs,
            tc.tile_pool(name="xcts", bufs=3) as xcts,
            tc.tile_pool(name="xctv", bufs=3) as xctv,
            tc.tile_pool(name="stats", bufs=6) as stats,
            tc.tile_pool(name="smalls", bufs=12) as smalls,
            tc.tile_pool(name="outs", bufs=3) as outs,
            tc.tile_pool(name="psum_t", bufs=3, space="PSUM") as psum_t,
            tc.tile_pool(name="psum_y", bufs=3, space="PSUM") as psum_y,
            tc.tile_pool(name="psum_d", bufs=1, space="PSUM") as psum_d,
        ):
            # ---- x loads: prefetch all 8 virtual tiles on the sync(SP)
            # ring.  Virtual tile v (c=v//2, j=v%2) holds DRAM rows
            # {256c + 2p + j}: one 2KB descriptor per partition at 4KB
            # stride, so a later store of tiles (2c, 2c+1) as [128,2,512]
            # needs only one contiguous 4KB descriptor per partition.
            x_all = []
            w2cs = consts.tile([P, 4, H], BF16)
            for v in range(NT):
                c, j = v // 2, v % 2
                xt = xs.tile([P, H], F32R)
                nc.sync.dma_start(
                    out=xt,
                    in_=x[c * 2 * P:(c + 1) * 2 * P, :].rearrange(
                        "(p two) k -> p two k", two=2)[:, j, :],
                )
                x_all.append(xt)

            # ---- one-time constants on the scalar(ACT) ring so they don't
            # delay x loads on the sync ring.  fp row (tiny) first: the ring
            # is FIFO and the first C-stage needs it.
            fpb_row = consts.tile([1, H], BF16)
            nc.scalar.dma_start(out=fpb_row, in_=_row_ap(fpb))
            ones_bf = consts.tile([1, P], BF16)
            nc.vector.memset(ones_bf, 1.0)

            # Identity build comes before the SWDGE weight loads: both run
            # on the Q7 and the first transposes are gated on the identity.
            iden_f32 = consts.tile([P, P], F32)
            make_identity(nc, iden_f32)
            iden = consts.tile([P, P], F32R)
            nc.gpsimd.tensor_copy(iden, iden_f32)

            # Per-chunk weight loads: chunk h must land before C-stage h
            # needs it.  Chunks 0/1 go through the GPSIMD SWDGE ring whose
            # descriptor generation runs on the otherwise-idle Q7, in
            # parallel with the HWDGE rings; 2/3 via the scalar ring.
            for h in range(4):
                eng = nc.gpsimd if h < 2 else nc.scalar
                eng.dma_start(out=w2cs[:, h, :], in_=w2c[h * P:(h + 1) * P, :])

            epst = consts.tile([P, 1], F32)
            nc.vector.memset(epst, EPS)

            # Hoist the ACT Sqrt table load into the preamble.
            dsq = smalls.tile([P, 1], F32, tag="dsq")
            nc.scalar.activation(dsq, epst, AF.Sqrt, bias=0.0, scale=1.0)

            # PE warm-up burst: keeps the tensor engine active while x0 is
            # in flight so HAM has up-clocked it before the real transposes.
            dwm = psum_d.tile([P, P], F32, tag="dwm")
            for _ in range(16):
                nc.tensor.matmul(dwm, ones_bf, ones_bf, start=True, stop=True)
            # Dummy PE op absorbing the identity's GPSIMD wait.
            d1 = psum_d.tile([P, P], F32R, tag="dummy")
            nc.tensor.transpose(d1, iden, iden)

            # ---- software-pipelined main loop over 8 token tiles ----
            # B(t): PE transposes tile t; evac adds fp chunk bias and casts
            #       to bf16 (2 chunks on ACT, 2 on DVE)
            # C(t-1): W2c matmuls + LN2 stats/apply
            # store chunk c after C(2c+1), interleaved like the loads.
            xct_all, ot_ch = {}, {}
            for i in range(NT + 1):
                if i < NT:
                    t = i
                    xt = x_all[t]
                    ptr = psum_t.tile([P, 4, P], F32R)
                    for h in range(4):
                        nc.tensor.transpose(ptr[:, h, :], xt[:, h * P:(h + 1) * P],
                                            iden)
                    xct_a = xcts.tile([P, 2, P], BF16)
                    nc.scalar.copy(xct_a, ptr[:, 0:2, :])
                    xct_b = xctv.tile([P, 2, P], BF16)
                    nc.vector.tensor_copy(xct_b, ptr[:, 2:4, :])
                    xct_all[t] = (xct_a, xct_b)

                if i >= 1:
                    k = i - 1
                    xct_a, xct_b = xct_all[k]
                    py = psum_y.tile([P, H], F32)
                    nc.tensor.matmul(py, ones_bf, fpb_row, start=True, stop=False)
                    for h in range(4):
                        src = xct_a[:, h, :] if h < 2 else xct_b[:, h - 2, :]
                        nc.tensor.matmul(
                            py, src, w2cs[:, h, :],
                            start=False, stop=(h == 3),
                        )

                    st2 = stats.tile([P, 6], F32, tag="st")
                    nc.vector.bn_stats(st2, py)
                    mv2 = stats.tile([P, 2], F32, tag="mv")
                    nc.vector.bn_aggr(mv2, st2)
                    sd2 = smalls.tile([P, 1], F32, tag="sd")
                    nc.scalar.activation(sd2, mv2[:, 1:2], AF.Sqrt, bias=epst,
                                         scale=1.0)
                    s2 = smalls.tile([P, 1], F32, tag="s")
                    nc.vector.reciprocal(s2, sd2)
                    negms2 = smalls.tile([P, 1], F32, tag="negms")
                    # Last two tiles: keep the chain on DVE — the GPSIMD hop
                    # adds ~300ns of latency to the pipeline drain.
                    neg_eng = nc.vector if k >= NT - 2 else nc.gpsimd
                    neg_eng.tensor_scalar(
                        negms2, mv2[:, 0:1], s2, -1.0, op0=ALU.mult, op1=ALU.mult
                    )

                    c, j = k // 2, k % 2
                    out_view = out[c * 2 * P:(c + 1) * 2 * P, :].rearrange(
                        "(p two) k -> p two k", two=2)
                    if k >= NT - 4:
                        # Drain region: store per tile so each store
                        # overlaps the next tile's LN2 chain and the final
                        # transfer is half the size -- these stores are the
                        # kernel's critical-path exit.
                        ots = outs.tile([P, H], F32, tag="single")
                        nc.scalar.activation(ots, py, AF.Identity,
                                             bias=negms2, scale=s2)
                        nc.sync.dma_start(out=out_view[:, j, :], in_=ots)
                    else:
                        if j == 0:
                            ot2 = outs.tile([P, 2, H], F32)
                            ot_ch[c] = ot2
                        nc.scalar.activation(ot_ch[c][:, j, :], py, AF.Identity,
                                             bias=negms2, scale=s2)
                        if j == 1:
                            nc.sync.dma_start(out=out_view, in_=ot_ch[c])

    nc.compile()
    return nc


# ---------------------------------------------------------------------------
# General (slow, fully-correct) program: original pipeline with LN1.
# Used only when c2 != 0 or g2 != 1 or b2 != 0.
# ---------------------------------------------------------------------------

def _bcast_ap(src: bass.AP, parts: int) -> bass.AP:
    return bass.AP(tensor=src.tensor, offset=src.offset, ap=[[0, parts]] + list(src.ap))


def build_general_program() -> bass.Bass:
    nc = bacc.Bacc("TRN2", target_bir_lowering=False, debug=False)

    x = nc.dram_tensor("x", [ROWS, H], F32, kind="ExternalInput").ap()
    w2 = nc.dram_tensor("w2", [H, H], F32, kind="ExternalInput").ap()   # [h,k]
    c2 = nc.dram_tensor("c2", [H], F32, kind="ExternalInput").ap()
    fp = nc.dram_tensor("fp", [H], F32, kind="ExternalInput").ap()
    g2 = nc.dram_tensor("g2", [H], F32, kind="ExternalInput").ap()
    b2 = nc.dram_tensor("b2", [H], F32, kind="ExternalInput").ap()
    out = nc.dram_tensor("out", [ROWS, H], F32, kind="ExternalOutput").ap()

    MD = F32R

    with tile.TileContext(nc) as tc:
        with (
            tc.tile_pool(name="consts", bufs=1) as consts,
            tc.tile_pool(name="xs", bufs=4) as xs,
            tc.tile_pool(name="zs", bufs=4) as zs,
            tc.tile_pool(name="xns", bufs=8) as xns,
            tc.tile_pool(name="xnts", bufs=3) as xnts,
            tc.tile_pool(name="stats", bufs=6) as stats,
            tc.tile_pool(name="smalls", bufs=12) as smalls,
            tc.tile_pool(name="ts", bufs=3) as ts_pool,
            tc.tile_pool(name="outs", bufs=3) as outs,
            tc.tile_pool(name="psum_t", bufs=3, space="PSUM") as psum_t,
            tc.tile_pool(name="psum_y", bufs=3, space="PSUM") as psum_y,
            tc.tile_pool(name="psum_d", bufs=1, space="PSUM") as psum_d,
        ):
            ones1 = consts.tile([1, P], F32)
            nc.vector.memset(ones1, 1.0)
            onesmm = consts.tile([1, P], MD)
            nc.vector.tensor_copy(onesmm, ones1)

            fprow = consts.tile([1, H], F32)
            nc.sync.dma_start(out=fprow, in_=_row_ap(fp))
            fpmm = consts.tile([1, H], MD)
            nc.vector.tensor_copy(fpmm, fprow)
            fp_ps = psum_d.tile([P, H], F32, tag="bcast")
            nc.tensor.matmul(fp_ps, onesmm, fpmm, start=True, stop=True)
            fpb = consts.tile([P, H], F32)
            nc.scalar.copy(fpb, fp_ps)

            g2b = consts.tile([P, H], F32)
            nc.gpsimd.dma_start(out=g2b, in_=_bcast_ap(g2, P))
            b2b = consts.tile([P, H], F32)
            nc.gpsimd.dma_start(out=b2b, in_=_bcast_ap(b2, P))

            c2row = consts.tile([1, H], F32)
            nc.sync.dma_start(out=c2row, in_=_row_ap(c2))
            c2mm = consts.tile([1, H], MD)
            nc.vector.tensor_copy(c2mm, c2row)

            iden_f32 = consts.tile([P, P], F32)
            make_identity(nc, iden_f32)
            iden = consts.tile([P, P], F32R)
            nc.gpsimd.tensor_copy(iden, iden_f32)
            epst = consts.tile([P, 1], F32)
            nc.vector.memset(epst, EPS)

            d1 = psum_d.tile([P, P], MD, tag="dummy")
            nc.tensor.transpose(d1, iden, iden)

            xn_all, xnt_all = {}, {}
            w2mm = consts.tile([P, 4, H], MD)
            for i in range(NT + 3):
                if i == 1:
                    w2s = consts.tile([P, 4, H], F32)
                    nc.sync.dma_start(
                        out=w2s, in_=w2.rearrange("(t p) k -> p t k", p=P)
                    )
                    nc.scalar.copy(w2mm, w2s)

                if i < NT:
                    xt = xs.tile([P, H], F32)
                    nc.sync.dma_start(out=xt, in_=x[i * P:(i + 1) * P, :])

                    z = zs.tile([P, H], F32)
                    nc.vector.tensor_add(z, xt, fpb)

                    st1 = stats.tile([P, 6], F32, tag="st")
                    nc.vector.bn_stats(st1, z)
                    mv1 = stats.tile([P, 2], F32, tag="mv")
                    nc.vector.bn_aggr(mv1, st1)
                    sd1 = smalls.tile([P, 1], F32, tag="sd")
                    nc.scalar.activation(sd1, mv1[:, 1:2], AF.Sqrt, bias=epst,
                                         scale=1.0)
                    s1 = smalls.tile([P, 1], F32, tag="s")
                    nc.vector.reciprocal(s1, sd1)
                    negms1 = smalls.tile([P, 1], F32, tag="negms")
                    nc.vector.tensor_scalar(
                        negms1, mv1[:, 0:1], s1, -1.0, op0=ALU.mult, op1=ALU.mult
                    )
                    xn = xns.tile([P, H], MD)
                    nc.scalar.activation(xn, z, AF.Identity, bias=negms1, scale=s1)
                    xn_all[i] = xn

                if 2 <= i < NT + 2:
                    j = i - 2
                    xn = xn_all[j]
                    ptr = psum_t.tile([P, 4, P], MD)
                    for h in range(4):
                        nc.tensor.transpose(ptr[:, h, :], xn[:, h * P:(h + 1) * P],
                                            iden)
                    xnt = xnts.tile([P, 4, P], MD)
                    nc.scalar.copy(xnt, ptr)
                    xnt_all[j] = xnt

                if i >= 3:
                    k = i - 3
                    xnt = xnt_all[k]
                    py = psum_y.tile([P, H], F32)
                    nc.tensor.matmul(py, onesmm, c2mm, start=True, stop=False)
                    for h in range(4):
                        nc.tensor.matmul(
                            py, xnt[:, h, :], w2mm[:, h, :],
                            start=False, stop=(h == 3),
                        )

                    st2 = stats.tile([P, 6], F32, tag="st")
                    nc.vector.bn_stats(st2, py)
                    mv2 = stats.tile([P, 2], F32, tag="mv")
                    nc.vector.bn_aggr(mv2, st2)
                    sd2 = smalls.tile([P, 1], F32, tag="sd")
                    nc.scalar.activation(sd2, mv2[:, 1:2], AF.Sqrt, bias=epst,
                                         scale=1.0)
                    s2 = smalls.tile([P, 1], F32, tag="s")
                    nc.vector.reciprocal(s2, sd2)
                    negms2 = smalls.tile([P, 1], F32, tag="negms")
                    nc.vector.tensor_scalar(
                        negms2, mv2[:, 0:1], s2, -1.0, op0=ALU.mult, op1=ALU.mult
                    )

                    t = ts_pool.tile([P, H], F32)
                    nc.scalar.activation(t, py, AF.Identity, bias=negms2, scale=s2)

                    t2 = outs.tile([P, H], F32, tag="t2")
                    nc.gpsimd.tensor_mul(t2, t, g2b)
                    ot = outs.tile([P, H], F32, tag="ot")
                    nc.gpsimd.tensor_add(ot, t2, b2b)

                    nc.sync.dma_start(out=out[k * P:(k + 1) * P, :], in_=ot)

    nc.compile()
    return nc


def _weights(Wo, g1, b1, bo):
    f32 = np.float32
    W2 = g1[:, None] * (Wo.T + np.eye(H, dtype=f32))           # [h,k]
    c2 = b1 + bo + Wo @ b1                                     # [k]
    return W2, c2


def _host_prep(temporal_features, static_features, Wt, bt, Wf, bf, Wo, bo,
               g1, b1, g2, b2):
    f32 = np.float32
    x = np.ascontiguousarray(np.asarray(temporal_features, dtype=f32)).reshape(B * S, H)
    st = np.asarray(static_features, dtype=f32)
    Wf = np.asarray(Wf, dtype=f32)
    bf = np.asarray(bf, dtype=f32)
    Wo = np.asarray(Wo, dtype=f32)
    bo = np.asarray(bo, dtype=f32)
    g1 = np.asarray(g1, dtype=f32)
    b1 = np.asarray(b1, dtype=f32)
    g2 = np.asarray(g2, dtype=f32)
    b2 = np.asarray(b2, dtype=f32)

    fp = st @ Wf.T + bf                                        # [B,H]
    W2, c2 = _weights(Wo, g1, b1, bo)

    fast = (not np.any(c2 != 0.0)) and (not np.any(g2 != 1.0)) \
        and (not np.any(b2 != 0.0))

    in_maps = []
    if fast:
        import ml_dtypes
        bf16 = np.dtype(ml_dtypes.bfloat16)
        wcs = W2.sum(axis=0)                                   # [k]
        W2c = W2 - wcs[None, :] / f32(H)                       # centered cols
        w2c_bf = np.ascontiguousarray(W2c.astype(bf16))
        fpw2c = (fp @ W2c).astype(f32)                          # [B,H]
        for c in range(N_CORES):
            shard = np.ascontiguousarray(x[c * ROWS:(c + 1) * ROWS])
            row = fpw2c[(c * ROWS) // S]
            in_maps.append({
                "x": shard,
                "w2c": w2c_bf,
                "fpb": np.ascontiguousarray(row.astype(bf16)),
            })
        return True, in_maps

    for c in range(N_CORES):
        shard = np.ascontiguousarray(x[c * ROWS:(c + 1) * ROWS])
        in_maps.append({
            "x": shard,
            "w2": np.ascontiguousarray(W2),
            "c2": np.ascontiguousarray(c2),
            "fp": np.ascontiguousarray(fp[(c * ROWS) // S]),
            "g2": np.ascontiguousarray(g2),
            "b2": np.ascontiguousarray(b2),
        })
    return False, in_maps


_NC_CACHE = {}


def _get_program(fast: bool):
    if fast not in _NC_CACHE:
        _NC_CACHE[fast] = build_fast_program() if fast else build_general_program()
    return _NC_CACHE[fast]


def run(inputs: dict, trace: bool = False):
    """Returns (output [B,S,H] f32, BassKernelResults)."""
    fast, in_maps = _host_prep(**inputs)
    nc = _get_program(fast)
    res = run_bass_kernel_spmd(nc, in_maps, list(range(N_CORES)), trace=trace)
    shards = [res.results[c]["out"] for c in range(N_CORES)]
    full = np.concatenate(shards, axis=0).reshape(B, S, H).astype(np.float32)
    return full, res


def kernel(**inputs) -> np.ndarray:
    out, _ = run(inputs, trace=False)
    return out
